# revision 4
# baseline (speedup 1.0000x reference)
"""Trainium2 Bass kernel for nn_Actor_35502199669063 (dense_mlp).

Network: x[65536,64] -> Linear(64,100)+LeakyReLU -> 100x(Linear(100,100)+LeakyReLU)
         -> Linear(100,1) -> tanh -> (a+1)/2*9+1

Strategy: pure data parallel over 8 NeuronCores (8192 rows each), parameters
replicated. Activations kept feature-major [features, batch] in SBUF so each
layer is one stationary-weight matmul streaming batch columns; biases folded in
via an appended ones-row (K=101). LeakyReLU is split across ScalarE (native
Lrelu) and VectorE. Matmuls run in float32r (full PE rate for fp32 storage).
"""

import numpy as np
from contextlib import ExitStack

import concourse.bass as bass
import concourse.bacc as bacc
import concourse.tile as tile
from concourse import mybir
from concourse.bass_utils import run_bass_kernel_spmd

# ---- problem constants (hardcoded; kernel.py must be self-contained) ----
B_TOTAL = 65536
N_OBS = 64
W = 100          # layer width
N_HIDDEN = 100   # number of hidden Ws
N_CORES = 8
B = B_TOTAL // N_CORES       # 8192 rows per core
ALPHA = 0.01
MIN_FREQ, MAX_FREQ = 1.0, 10.0
OUT_SCALE = (MAX_FREQ - MIN_FREQ) / 2.0   # 4.5
OUT_BIAS = (MAX_FREQ + MIN_FREQ) / 2.0    # 5.5

N_TILE = 512                  # matmul moving-dim / one PSUM bank of fp32
GROUP_COLS = 2048             # 4 banks per pipeline group
N_GROUPS = B // GROUP_COLS    # 4 groups per layer
ACT_COLS = 1536               # banks 0-2 of each group -> ScalarE Lrelu
DVE_COLS = GROUP_COLS - ACT_COLS  # bank 3 -> VectorE two-op leaky

FP32 = mybir.dt.float32
# matmul-feeding tensors are declared float32r: 4-byte fp32 storage that the PE
# streams at full rate (plain fp32 matmul runs at 1/4 rate). numpy side is float32.
MMDT = mybir.dt.float32r


def build_nc(repeats=1):
    nc = bacc.Bacc("TRN2", target_bir_lowering=False, debug=False)

    xt_ext = nc.declare_dram_parameter("xt", [N_OBS + 1, B], MMDT, isOutput=False)
    ws_ext = nc.declare_dram_parameter("ws", [W + 1, N_HIDDEN * W], MMDT, isOutput=False)
    wi_ext = nc.declare_dram_parameter("wi", [N_OBS + 1, W], MMDT, isOutput=False)
    wo_ext = nc.declare_dram_parameter("wo", [W + 1, 1], MMDT, isOutput=False)
    out_ext = nc.declare_dram_parameter("out", [1, B], FP32, isOutput=True)

    with tile.TileContext(nc) as tc, ExitStack() as ctx:
        cpool = ctx.enter_context(tc.tile_pool(name="cpool", bufs=1))

        xt = cpool.tile([N_OBS + 1, B], MMDT)
        nc.sync.dma_start(xt[:], xt_ext[:])
        ws = cpool.tile([W + 1, N_HIDDEN * W], MMDT)
        nc.sync.dma_start(ws[:], ws_ext[:])
        wi = cpool.tile([N_OBS + 1, W], MMDT)
        nc.sync.dma_start(wi[:], wi_ext[:])
        wo = cpool.tile([W + 1, 1], MMDT)
        nc.sync.dma_start(wo[:], wo_ext[:])

        hA = cpool.tile([W + 1, B], MMDT)
        hB = cpool.tile([W + 1, B], MMDT)
        # ones rows for the bias trick (copied from xt's ones row in DRAM)
        nc.sync.dma_start(hA[W : W + 1, :], xt_ext[N_OBS : N_OBS + 1, :])
        nc.sync.dma_start(hB[W : W + 1, :], xt_ext[N_OBS : N_OBS + 1, :])

        out_sb = cpool.tile([1, B], FP32)

        for _rep in range(repeats):
          with (
            tc.tile_pool(name="psA", bufs=2, space="PSUM") as psA,
            tc.tile_pool(name="psB", bufs=2, space="PSUM") as psB,
            tc.tile_pool(name="dtmp", bufs=2) as dtmp,
          ):
            # layers 0..N_HIDDEN: l=0 is the input layer (K=65), 1..100 hidden
            for l in range(N_HIDDEN + 1):
                # ping-pong: l=0 reads xt, writes hA; odd l reads hA writes hB;
                # even l>=2 reads hB writes hA. (l=100 lands in hA.)
                if l == 0:
                    src, dst, w_ap = xt, hA, wi[:]
                elif l % 2 == 1:
                    src, dst, w_ap = hA, hB, ws[:, (l - 1) * W : l * W]
                else:
                    src, dst, w_ap = hB, hA, ws[:, (l - 1) * W : l * W]

                for g in range(N_GROUPS):
                    g0 = g * GROUP_COLS
                    zA = psA.tile([W, ACT_COLS], FP32, tag="zA")
                    for k in range(ACT_COLS // N_TILE):
                        c0 = g0 + k * N_TILE
                        nc.tensor.matmul(
                            zA[:, k * N_TILE : (k + 1) * N_TILE],
                            w_ap,
                            src[:, c0 : c0 + N_TILE],
                            start=True,
                            stop=True,
                        )
                    zB = psB.tile([W, DVE_COLS], FP32, tag="zB")
                    for k in range(DVE_COLS // N_TILE):
                        c0 = g0 + ACT_COLS + k * N_TILE
                        nc.tensor.matmul(
                            zB[:, k * N_TILE : (k + 1) * N_TILE],
                            w_ap,
                            src[:, c0 : c0 + N_TILE],
                            start=True,
                            stop=True,
                        )
                    # ScalarE native leaky on banks 0-2
                    nc.scalar.activation(
                        dst[0:W, g0 : g0 + ACT_COLS],
                        zA[:],
                        mybir.ActivationFunctionType.Lrelu,
                        alpha=ALPHA,
                    )
                    # VectorE two-op leaky on bank 3
                    t = dtmp.tile([W, DVE_COLS], FP32, tag="t")
                    nc.vector.tensor_scalar(t[:], zB[:], ALPHA, None, mybir.AluOpType.mult)
                    nc.vector.tensor_tensor(
                        dst[0:W, g0 + ACT_COLS : g0 + GROUP_COLS],
                        zB[:],
                        t[:],
                        mybir.AluOpType.max,
                    )

          # output layer: z = wo.T @ h_last; out = tanh(z)*4.5 + 5.5
          h_last = hA if (N_HIDDEN % 2 == 0) else hB
          with tc.tile_pool(name="psO", bufs=2, space="PSUM") as psO:
            for q in range(N_GROUPS):
                q0 = q * GROUP_COLS
                zo = psO.tile([1, GROUP_COLS], FP32, tag="zo")
                for k in range(GROUP_COLS // N_TILE):
                    c0 = q0 + k * N_TILE
                    nc.tensor.matmul(
                        zo[:, k * N_TILE : (k + 1) * N_TILE],
                        wo[:],
                        h_last[:, c0 : c0 + N_TILE],
                        start=True,
                        stop=True,
                    )
                nc.scalar.activation(
                    out_sb[0:1, q0 : q0 + GROUP_COLS],
                    zo[:],
                    mybir.ActivationFunctionType.Tanh,
                )
            nc.vector.tensor_scalar(
                out_sb[:], out_sb[:], OUT_SCALE, OUT_BIAS,
                mybir.AluOpType.mult, mybir.AluOpType.add,
            )

        nc.sync.dma_start(out_ext[:], out_sb[:])

    nc.compile()
    return nc


_NC_CACHE = {}


def get_nc(repeats=1):
    key = ("nc", repeats)
    if key not in _NC_CACHE:
        _NC_CACHE[key] = build_nc(repeats)
    return _NC_CACHE[key]


def make_in_maps(x, W_in, b_in, Ws, bs, W_out, b_out):
    """Host-side prep: shard/transpose/augment. All fp32."""
    x = np.ascontiguousarray(x, dtype=np.float32)
    ws_host = np.empty((W + 1, N_HIDDEN * W), dtype=np.float32)
    for i in range(N_HIDDEN):
        ws_host[0:W, i * W : (i + 1) * W] = Ws[i]
        ws_host[W, i * W : (i + 1) * W] = bs[i]
    wi_host = np.empty((N_OBS + 1, W), dtype=np.float32)
    wi_host[0:N_OBS] = W_in
    wi_host[N_OBS] = b_in
    wo_host = np.empty((W + 1, 1), dtype=np.float32)
    wo_host[0:W] = np.asarray(W_out, dtype=np.float32).reshape(W, 1)
    wo_host[W] = np.float32(b_out).reshape(1)

    in_maps = []
    for c in range(N_CORES):
        shard = x[c * B : (c + 1) * B]          # [B, 64]
        xt_host = np.empty((N_OBS + 1, B), dtype=np.float32)
        xt_host[0:N_OBS] = shard.T
        xt_host[N_OBS] = 1.0
        in_maps.append(
            {"xt": xt_host, "ws": ws_host, "wi": wi_host, "wo": wo_host}
        )
    return in_maps


def kernel(x, W_in, b_in, Ws, bs, W_out, b_out):
    nc = get_nc()
    in_maps = make_in_maps(x, W_in, b_in, Ws, bs, W_out, b_out)
    res = run_bass_kernel_spmd(nc, in_maps, core_ids=list(range(N_CORES)))
    out = np.empty((B_TOTAL, 1), dtype=np.float32)
    for c in range(N_CORES):
        out[c * B : (c + 1) * B, 0] = res.results[c]["out"].reshape(B)
    return out


# revision 8
# speedup vs baseline: 1.0319x; 1.0319x over previous
"""Trainium2 Bass kernel for nn_Actor_35502199669063 (dense_mlp).

Network: x[65536,64] -> Linear(64,100)+LeakyReLU -> 100x(Linear(100,100)+LeakyReLU)
         -> Linear(100,1) -> tanh -> (a+1)/2*9+1

Strategy: pure data parallel over 8 NeuronCores (8192 rows each), parameters
replicated. Activations kept feature-major [features, batch] in SBUF so each
layer is one stationary-weight matmul streaming batch columns; biases folded in
via an appended ones-row (K=101). LeakyReLU is split across ScalarE (native
Lrelu) and VectorE. Matmuls run in float32r (full PE rate for fp32 storage).
"""

import numpy as np
from contextlib import ExitStack

import concourse.bass as bass
import concourse.bacc as bacc
import concourse.tile as tile
from concourse import mybir
from concourse.bass_utils import run_bass_kernel_spmd

# ---- problem constants (hardcoded; kernel.py must be self-contained) ----
B_TOTAL = 65536
N_OBS = 64
W = 100          # layer width
N_HIDDEN = 100   # number of hidden Ws
N_CORES = 8
B = B_TOTAL // N_CORES       # 8192 rows per core
ALPHA = 0.01
MIN_FREQ, MAX_FREQ = 1.0, 10.0
OUT_SCALE = (MAX_FREQ - MIN_FREQ) / 2.0   # 4.5
OUT_BIAS = (MAX_FREQ + MIN_FREQ) / 2.0    # 5.5

N_TILE = 512                  # matmul moving-dim / one PSUM bank of fp32
GROUP_COLS = 2048             # 4 banks per pipeline group
N_GROUPS = B // GROUP_COLS    # 4 groups per layer
ACT_COLS = 1024               # banks 0-1 of each group -> ScalarE Lrelu
DVE_COLS = GROUP_COLS - ACT_COLS  # banks 2-3 -> VectorE fused leaky

FP32 = mybir.dt.float32
# matmul-feeding tensors are declared float32r: 4-byte fp32 storage that the PE
# streams at full rate (plain fp32 matmul runs at 1/4 rate). numpy side is float32.
MMDT = mybir.dt.float32r


def _register_leaky_op():
    """Register a fused one-instruction DVE leaky-relu: out = max(in0*s0, in0).

    Stock DVE needs two instructions (tensor_scalar mult + tensor_tensor max),
    halving VectorE's effective PSUM-drain rate; this custom op restores 1x.
    """
    from concourse import dve_ops
    from concourse.dve_spec import Spec, Src0, C0, maxx, lower, _has_src1
    from concourse.dve_uop import DveOpSpec

    name = "LRELU_ANT"
    for op in dve_ops.OPS:
        if op.name == name:
            return op
    spec = Spec(
        body=maxx(Src0 * C0, Src0),
        reference=lambda in0, in1, s0, s1, imm2: np.maximum(
            in0.astype(np.float32) * np.float32(s0), in0.astype(np.float32)
        ),
    )
    row = dve_ops._CUSTOM_DVE_ROW_BASE + len(dve_ops.OPS)
    assert row < 0x20
    dve_ops._SUB_OPCODE_FOR_NAME[name] = row
    shas = {}
    for ver in ("v3", "v4"):
        tmp = DveOpSpec(name=name, opcode=row, uops=lower(spec, ver=ver),
                        rd1_en=_has_src1(spec))
        shas[ver] = tmp.sha(ver)
    op = dve_ops.DveOp(name, spec, subdim=False, uops_sha=shas)
    dve_ops.OPS.append(op)
    dve_ops.CUSTOM_DVE_SPECS[name] = spec
    return op


LRELU_DVE = _register_leaky_op()


def build_nc(repeats=1):
    nc = bacc.Bacc("TRN2", target_bir_lowering=False, debug=False)

    xt_ext = nc.declare_dram_parameter("xt", [N_OBS + 1, B], MMDT, isOutput=False)
    ws_ext = nc.declare_dram_parameter("ws", [W + 1, N_HIDDEN * W], MMDT, isOutput=False)
    wi_ext = nc.declare_dram_parameter("wi", [N_OBS + 1, W], MMDT, isOutput=False)
    wo_ext = nc.declare_dram_parameter("wo", [W + 1, 1], MMDT, isOutput=False)
    out_ext = nc.declare_dram_parameter("out", [1, B], FP32, isOutput=True)

    with tile.TileContext(nc) as tc, ExitStack() as ctx:
        cpool = ctx.enter_context(tc.tile_pool(name="cpool", bufs=1))

        xt = cpool.tile([N_OBS + 1, B], MMDT)
        nc.sync.dma_start(xt[:], xt_ext[:])
        ws = cpool.tile([W + 1, N_HIDDEN * W], MMDT)
        nc.sync.dma_start(ws[:], ws_ext[:])
        wi = cpool.tile([N_OBS + 1, W], MMDT)
        nc.sync.dma_start(wi[:], wi_ext[:])
        wo = cpool.tile([W + 1, 1], MMDT)
        nc.sync.dma_start(wo[:], wo_ext[:])

        hA = cpool.tile([W + 1, B], MMDT)
        hB = cpool.tile([W + 1, B], MMDT)
        # ones rows for the bias trick (copied from xt's ones row in DRAM)
        nc.sync.dma_start(hA[W : W + 1, :], xt_ext[N_OBS : N_OBS + 1, :])
        nc.sync.dma_start(hB[W : W + 1, :], xt_ext[N_OBS : N_OBS + 1, :])

        out_sb = cpool.tile([1, B], FP32)

        for _rep in range(repeats):
          with (
            tc.tile_pool(name="psA", bufs=2, space="PSUM") as psA,
            tc.tile_pool(name="psB", bufs=2, space="PSUM") as psB,
          ):
            # layers 0..N_HIDDEN: l=0 is the input layer (K=65), 1..100 hidden
            for l in range(N_HIDDEN + 1):
                # ping-pong: l=0 reads xt, writes hA; odd l reads hA writes hB;
                # even l>=2 reads hB writes hA. (l=100 lands in hA.)
                if l == 0:
                    src, dst, w_ap = xt, hA, wi[:]
                elif l % 2 == 1:
                    src, dst, w_ap = hA, hB, ws[:, (l - 1) * W : l * W]
                else:
                    src, dst, w_ap = hB, hA, ws[:, (l - 1) * W : l * W]

                for g in range(N_GROUPS):
                    g0 = g * GROUP_COLS
                    zA = psA.tile([W, ACT_COLS], FP32, tag="zA")
                    for k in range(ACT_COLS // N_TILE):
                        c0 = g0 + k * N_TILE
                        nc.tensor.matmul(
                            zA[:, k * N_TILE : (k + 1) * N_TILE],
                            w_ap,
                            src[:, c0 : c0 + N_TILE],
                            start=True,
                            stop=True,
                        )
                    zB = psB.tile([W, DVE_COLS], FP32, tag="zB")
                    for k in range(DVE_COLS // N_TILE):
                        c0 = g0 + ACT_COLS + k * N_TILE
                        nc.tensor.matmul(
                            zB[:, k * N_TILE : (k + 1) * N_TILE],
                            w_ap,
                            src[:, c0 : c0 + N_TILE],
                            start=True,
                            stop=True,
                        )
                    # ScalarE native leaky on banks 0-2
                    nc.scalar.activation(
                        dst[0:W, g0 : g0 + ACT_COLS],
                        zA[:],
                        mybir.ActivationFunctionType.Lrelu,
                        alpha=ALPHA,
                    )
                    # VectorE fused one-op leaky on banks 2-3
                    nc.vector._custom_dve(
                        LRELU_DVE,
                        out=dst[0:W, g0 + ACT_COLS : g0 + GROUP_COLS],
                        in0=zB[:],
                        s0=ALPHA,
                    )

          # output layer: z = wo.T @ h_last; out = tanh(z)*4.5 + 5.5
          h_last = hA if (N_HIDDEN % 2 == 0) else hB
          with tc.tile_pool(name="psO", bufs=2, space="PSUM") as psO:
            for q in range(N_GROUPS):
                q0 = q * GROUP_COLS
                zo = psO.tile([1, GROUP_COLS], FP32, tag="zo")
                for k in range(GROUP_COLS // N_TILE):
                    c0 = q0 + k * N_TILE
                    nc.tensor.matmul(
                        zo[:, k * N_TILE : (k + 1) * N_TILE],
                        wo[:],
                        h_last[:, c0 : c0 + N_TILE],
                        start=True,
                        stop=True,
                    )
                nc.scalar.activation(
                    out_sb[0:1, q0 : q0 + GROUP_COLS],
                    zo[:],
                    mybir.ActivationFunctionType.Tanh,
                )
            nc.vector.tensor_scalar(
                out_sb[:], out_sb[:], OUT_SCALE, OUT_BIAS,
                mybir.AluOpType.mult, mybir.AluOpType.add,
            )

        nc.sync.dma_start(out_ext[:], out_sb[:])

    nc.compile()
    return nc


_NC_CACHE = {}


def get_nc(repeats=1):
    key = ("nc", repeats)
    if key not in _NC_CACHE:
        _NC_CACHE[key] = build_nc(repeats)
    return _NC_CACHE[key]


def make_in_maps(x, W_in, b_in, Ws, bs, W_out, b_out):
    """Host-side prep: shard/transpose/augment. All fp32."""
    x = np.ascontiguousarray(x, dtype=np.float32)
    ws_host = np.empty((W + 1, N_HIDDEN * W), dtype=np.float32)
    for i in range(N_HIDDEN):
        ws_host[0:W, i * W : (i + 1) * W] = Ws[i]
        ws_host[W, i * W : (i + 1) * W] = bs[i]
    wi_host = np.empty((N_OBS + 1, W), dtype=np.float32)
    wi_host[0:N_OBS] = W_in
    wi_host[N_OBS] = b_in
    wo_host = np.empty((W + 1, 1), dtype=np.float32)
    wo_host[0:W] = np.asarray(W_out, dtype=np.float32).reshape(W, 1)
    wo_host[W] = np.float32(b_out).reshape(1)

    in_maps = []
    for c in range(N_CORES):
        shard = x[c * B : (c + 1) * B]          # [B, 64]
        xt_host = np.empty((N_OBS + 1, B), dtype=np.float32)
        xt_host[0:N_OBS] = shard.T
        xt_host[N_OBS] = 1.0
        in_maps.append(
            {"xt": xt_host, "ws": ws_host, "wi": wi_host, "wo": wo_host}
        )
    return in_maps


def kernel(x, W_in, b_in, Ws, bs, W_out, b_out):
    nc = get_nc()
    in_maps = make_in_maps(x, W_in, b_in, Ws, bs, W_out, b_out)
    res = run_bass_kernel_spmd(nc, in_maps, core_ids=list(range(N_CORES)))
    out = np.empty((B_TOTAL, 1), dtype=np.float32)
    for c in range(N_CORES):
        out[c * B : (c + 1) * B, 0] = res.results[c]["out"].reshape(B)
    return out
